# revision 1
# baseline (speedup 1.0000x reference)
"""Routed-LoRA linear layer (moe_routing) on 8 trn2 NeuronCores.

Math (per token t):
  out[t, :] = W @ x[t] + b + 2.0 * sum_n mask[n, t] * (B_n @ (A_n @ x[t]))

Strategy:
  - Data-parallel over B*T = 65536 tokens: 8192 tokens per core.
  - Streaming operands are marshaled to bf16 host-side: halves HBM traffic
    and SBUF footprint; output error ~3e-3 relative, well inside the 2e-2
    gate. PSUM accumulation stays fp32.
  - Host-side transposes give the device contiguous, partition-friendly
    layouts only:
      xt   [D_IN, TOK]   = x-shard transposed (contraction dim major)
      wt   [D_IN, D_OUT] = W.T
      at   [P, KC*NR]    = fused-A.T, pre-packed per partition
      btr  [NR, D_OUT]   = fused-B.T
      mtok [P, G, N]     = per-token routing mask, token-partition layout
  - The LoRA s = A@x projection rides the main matmul's M-tiles as a third
    small N=64 matmul per contraction chunk (2 PE cyc/token instead of 8
    for a separate [NR,SUP]-layout pass), is masked on DVE with a stride-0
    rank-broadcast AP, PE-transposed back to rank-partition layout, and
    accumulated into the base matmul's PSUM bank as a 9th contraction
    chunk. Bias is added during the PSUM->SBUF copy; output is stored
    bf16 and upcast on host.
  - Supertile 0 runs k-outer across two q-tiles so the PE consumes each
    arriving (x-chunk, W-chunk) DMA pair slower than the stream delivers;
    per-128-token output stores keep the drain tail short.
"""

import numpy as np
import ml_dtypes

import concourse.bass as bass
from concourse import bacc
from concourse.masks import make_identity
import concourse.mybir as mybir
import concourse.tile as tile
from concourse.bass_utils import run_bass_kernel_spmd

N_CORES = 8
B, T = 8, 8192
D_IN = 1024
D_OUT = 1024
N_ADAPT, R = 4, 16
NR = N_ADAPT * R  # 64
SCALING = 32.0 / 16.0

TOK = B * T // N_CORES  # 8192 tokens per core
SUP = 512               # tokens per supertile
N_SUP = TOK // SUP      # 16
SUB = 128               # tokens per matmul M-tile
N_SUB = SUP // SUB      # 4
G = N_SUP * N_SUB       # 64 M-tiles per core
P = 128
KC = D_IN // P          # 8 contraction chunks
NB = D_OUT // 512       # 2 PSUM-bank column halves

F32 = mybir.dt.float32
BF16 = mybir.dt.bfloat16
NP_BF16 = ml_dtypes.bfloat16


def build_bass(xp_bufs=4, op_bufs=6, pso_bufs=6):
    nc = bacc.Bacc(
        "TRN2", target_bir_lowering=False, debug=False, num_devices=N_CORES
    )

    xt_d = nc.dram_tensor("xt", [D_IN, TOK], BF16, kind="ExternalInput")
    wt_d = nc.dram_tensor("wt", [D_IN, D_OUT], BF16, kind="ExternalInput")
    at_d = nc.dram_tensor("at", [P, KC * NR], BF16, kind="ExternalInput")
    bt_d = nc.dram_tensor("btr", [NR, D_OUT], BF16, kind="ExternalInput")
    bias_d = nc.dram_tensor("bias", [D_OUT], BF16, kind="ExternalInput")
    mtok_d = nc.dram_tensor("mtok", [P, G * N_ADAPT], BF16, kind="ExternalInput")
    out_d = nc.dram_tensor("out", [TOK, D_OUT], BF16, kind="ExternalOutput")

    xt_r = xt_d.ap().rearrange("(kc p) t -> p kc t", p=P)
    wt_r = wt_d.ap().rearrange("(kc p) n -> p kc n", p=P)
    out_r = out_d.ap().rearrange("(s q p) n -> s q p n", q=N_SUB, p=P)
    bias_bcast = bass.AP(
        tensor=bias_d, offset=0, ap=[[0, P], [1, D_OUT]]
    )

    with tile.TileContext(nc) as tc:
        with (
            tc.tile_pool(name="const", bufs=1) as const,
            tc.tile_pool(name="xp", bufs=xp_bufs) as xp,
            tc.tile_pool(name="smqp", bufs=2) as smqp,
            tc.tile_pool(name="smtp", bufs=2) as smtp,
            tc.tile_pool(name="op", bufs=op_bufs) as op,
            tc.tile_pool(name="pso", bufs=pso_bufs, space="PSUM") as pso,
            tc.tile_pool(name="psj", bufs=1, space="PSUM") as psj,
            tc.tile_pool(name="pst", bufs=1, space="PSUM") as pst,
        ):
            w_sb = const.tile([P, KC, D_OUT], BF16)
            a_sb = const.tile([P, KC, NR], BF16)
            bt_sb = const.tile([NR, D_OUT], BF16)
            b_sb = const.tile([P, D_OUT], BF16)
            m_sb = const.tile([P, G, N_ADAPT], BF16)
            ident = const.tile([P, P], BF16)
            make_identity(nc, ident[:])
            # Preload order matters for startup latency: the first matmuls
            # need a_sb + x0 chunk 0 (sync queue) and W chunk k in order
            # (scalar queue); everything else is needed later.
            for k in range(KC):
                nc.scalar.dma_start(out=w_sb[:, k, :], in_=wt_r[:, k, :])
            nc.scalar.dma_start(out=bt_sb[:], in_=bt_d.ap())
            nc.scalar.dma_start(out=b_sb[:], in_=bias_bcast)
            # a/m ride the back of the scalar preload queue: in place well
            # before the post-k-outer sj/mask phase needs them (~16us), and
            # never ahead of x0/W in the serialized DMA stream
            nc.scalar.dma_start(
                out=a_sb[:],
                in_=at_d.ap().rearrange("p (kc j) -> p kc j", kc=KC),
            )
            nc.scalar.dma_start(
                out=m_sb[:],
                in_=mtok_d.ap().rearrange("p (g n) -> p g n", g=G),
            )

            for s in range(N_SUP):
                t0 = s * SUP
                x_sb = xp.tile([P, KC, SUP], BF16, tag="x")
                if s == 0:
                    # chunked first load: consumers of chunk k can start as
                    # soon as chunk k lands instead of after the full load
                    for k in (0, 1):
                        nc.sync.dma_start(
                            out=x_sb[:, k, :], in_=xt_r[:, k, t0 : t0 + SUP]
                        )
                    for k in range(2, KC, 2):
                        nc.sync.dma_start(
                            out=x_sb[:, k : k + 2, :],
                            in_=xt_r[:, k : k + 2, t0 : t0 + SUP],
                        )
                else:
                    nc.sync.dma_start(
                        out=x_sb[:], in_=xt_r[:, :, t0 : t0 + SUP]
                    )

                # per-q-tile state.  PSUM tiles are padded to a full 2KB
                # bank: accumulation-group `start` clears the whole bank, so
                # a bank must never host two in-flight groups.
                sj = {}    # [P, 64] fp32 s-projection (PSUM, bank-padded)
                smq = {}   # [P, 64] bf16 masked s, token-partition (SBUF)
                smt = {}   # [64, 128] bf16 masked s.T via PE transpose
                smt_sb = {}

                def sj_block(q):
                    # s[tok, j] += x-chunk.T @ A-chunk, rides the M-tile
                    sj[q] = psj.tile([P, 512], F32, tag="sj", name=f"sj{q}")
                    for k in range(KC):
                        nc.tensor.matmul(
                            sj[q][:, :NR],
                            x_sb[:, k, q * SUB : (q + 1) * SUB],
                            a_sb[:, k, :],
                            start=(k == 0),
                            stop=(k == KC - 1),
                        )

                def mask_q(q):
                    # rank-broadcast the per-adapter mask along r via a
                    # stride-0 AP: j = n*R + r
                    smq[q] = smqp.tile([P, NR], BF16, tag="smq", name=f"smq{q}")
                    m_bc = (
                        m_sb[:, s * N_SUB + q, :]
                        .unsqueeze(2)
                        .broadcast_to((P, N_ADAPT, R))
                    )
                    nc.vector.tensor_mul(
                        smq[q][:].rearrange("p (n r) -> p n r", n=N_ADAPT),
                        sj[q][:, :NR].rearrange("p (n r) -> p n r", n=N_ADAPT),
                        m_bc,
                    )

                def transpose_q(q):
                    smt[q] = pst.tile([NR, 1024], BF16, tag="smt", name=f"smt{q}")
                    nc.tensor.transpose(
                        smt[q][:, :SUB], smq[q][:], ident[:]
                    )

                def copy_q(q):
                    smt_sb[q] = smtp.tile(
                        [NR, SUB], BF16, tag="smtsb", name=f"smtsb{q}"
                    )
                    nc.vector.tensor_copy(smt_sb[q][:], smt[q][:, :SUB])

                def main_half(q, n, o_ps_h, skip=False):
                    ts = q * SUB
                    nsl = slice(n * 512, (n + 1) * 512)
                    for k in range(KC):
                        nc.tensor.matmul(
                            o_ps_h[:],
                            x_sb[:, k, ts : ts + SUB],
                            w_sb[:, k, nsl],
                            start=(k == 0),
                            stop=False,
                            skip_group_check=skip,
                        )

                def lora_half(q, n, o_ps_h, skip=False):
                    nsl = slice(n * 512, (n + 1) * 512)
                    nc.tensor.matmul(
                        o_ps_h[:],
                        smt_sb[q][:],
                        bt_sb[:, nsl],
                        start=False,
                        stop=True,
                        skip_group_check=skip,
                    )
                    o_sb = op.tile([P, 512], BF16, tag="o")
                    nc.vector.tensor_add(o_sb[:], o_ps_h[:], b_sb[:, nsl])
                    # the closing store rides the idle sync queue so its
                    # descriptor generation overlaps the scalar queue's
                    eng = nc.sync if (s == N_SUP - 1 and q == N_SUB - 1
                                      and n == NB - 1) else nc.scalar
                    eng.dma_start(out=out_r[s, q][:, nsl], in_=o_sb[:])

                if s == 0:
                    # Startup: k-outer across q0/q1 mains so each arriving
                    # (x-chunk, W-chunk) DMA pair unlocks PE work faster
                    # than the serialized preload stream delivers it.
                    ph01 = {}
                    for q in (0, 1, 2):
                        for n in range(NB):
                            ph01[q, n] = pso.tile(
                                [P, 512], F32, tag="ops", name=f"ops01_{q}_{n}"
                            )
                    for k in range(KC):
                        for q in (0, 1, 2):
                            for n in range(NB):
                                nsl = slice(n * 512, (n + 1) * 512)
                                nc.tensor.matmul(
                                    ph01[q, n][:],
                                    x_sb[:, k, q * SUB : (q + 1) * SUB],
                                    w_sb[:, k, nsl],
                                    start=(k == 0),
                                    stop=False,
                                    skip_group_check=True,
                                )
                    for q in range(N_SUB):
                        sj_block(q)
                        mask_q(q)
                    for q in range(N_SUB):
                        transpose_q(q)
                        copy_q(q)
                    for q in (0, 1, 2):
                        for n in range(NB):
                            lora_half(q, n, ph01[q, n], skip=True)
                    for q in (3,):
                        o_ps = {}
                        for n in range(NB):
                            o_ps[n] = pso.tile(
                                [P, 512], F32, tag="ops", name=f"ops0_{q}_{n}"
                            )
                            main_half(q, n, o_ps[n])
                        for n in range(NB):
                            lora_half(q, n, o_ps[n])
                else:
                    # Steady state: sj/transpose/copy for tile q run early,
                    # interleaved with the q-1/q main matmuls, so the LoRA-B
                    # matmul never waits on the DVE round trip.
                    o_ps = {}
                    sj_block(0)
                    mask_q(0)
                    o_ps[0, 0] = pso.tile([P, 512], F32, tag="ops", name="opsA")
                    main_half(0, 0, o_ps[0, 0])
                    sj_block(1)
                    mask_q(1)
                    transpose_q(0)
                    copy_q(0)
                    o_ps[0, 1] = pso.tile([P, 512], F32, tag="ops", name="opsB")
                    main_half(0, 1, o_ps[0, 1])
                    lora_half(0, 0, o_ps[0, 0])
                    lora_half(0, 1, o_ps[0, 1])
                    for q in (1, 2):
                        sj_block(q + 1)
                        mask_q(q + 1)
                        transpose_q(q)
                        copy_q(q)
                        for n in range(NB):
                            o_ps[q, n] = pso.tile(
                                [P, 512], F32, tag="ops", name=f"ops_{q}_{n}"
                            )
                            main_half(q, n, o_ps[q, n])
                        for n in range(NB):
                            lora_half(q, n, o_ps[q, n])
                    transpose_q(3)
                    copy_q(3)
                    for n in range(NB):
                        o_ps[3, n] = pso.tile(
                            [P, 512], F32, tag="ops", name=f"ops_3_{n}"
                        )
                        main_half(3, n, o_ps[3, n])
                    for n in range(NB):
                        lora_half(3, n, o_ps[3, n])

    nc.compile()
    return nc


_NC_CACHE = None


def _get_nc():
    global _NC_CACHE
    if _NC_CACHE is None:
        _NC_CACHE = build_bass()
    return _NC_CACHE


def make_in_maps(x, W, b, lora_A, lora_B, masks):
    x = np.ascontiguousarray(x, dtype=np.float32)
    W = np.ascontiguousarray(W, dtype=np.float32)
    b = np.ascontiguousarray(b, dtype=np.float32)
    lora_A = np.ascontiguousarray(lora_A, dtype=np.float32)
    lora_B = np.ascontiguousarray(lora_B, dtype=np.float32)
    masks = np.ascontiguousarray(masks, dtype=np.float32)

    x_flat = x.reshape(B * T, D_IN)
    A_flat = lora_A.reshape(NR, D_IN)
    B_flat = lora_B.transpose(1, 0, 2).reshape(D_OUT, NR)

    wt = np.ascontiguousarray(W.T.astype(NP_BF16))       # [D_IN, D_OUT]
    # packed [P, KC*NR]: per-partition contiguous 1KB rows (full DMA rate)
    at = np.ascontiguousarray(
        A_flat.T.astype(NP_BF16).reshape(KC, P, NR).transpose(1, 0, 2)
        .reshape(P, KC * NR)
    )
    btr = np.ascontiguousarray(B_flat.T.astype(NP_BF16))  # [NR, D_OUT]

    # per-token mask, token-partition layout [P, G*N_ADAPT]
    m_full = masks[..., 0].reshape(N_ADAPT, B * T) * np.float32(SCALING)

    in_maps = []
    for c in range(N_CORES):
        sl = slice(c * TOK, (c + 1) * TOK)
        mtok = np.ascontiguousarray(
            m_full[:, sl].T.astype(NP_BF16)             # [TOK, N]
            .reshape(G, P, N_ADAPT).transpose(1, 0, 2)  # [P, G, N]
            .reshape(P, G * N_ADAPT)
        )
        in_maps.append(
            {
                "xt": np.ascontiguousarray(x_flat[sl].astype(NP_BF16).T),
                "wt": wt,
                "at": at,
                "btr": btr,
                "bias": b.astype(NP_BF16),
                "mtok": mtok,
            }
        )
    return in_maps


def kernel(x, W, b, lora_A, lora_B, masks):
    nc = _get_nc()
    in_maps = make_in_maps(x, W, b, lora_A, lora_B, masks)
    res = run_bass_kernel_spmd(nc, in_maps, core_ids=list(range(N_CORES)))
    out = np.concatenate([r["out"] for r in res.results], axis=0)
    out = out.astype(np.float32).reshape(B, T, D_OUT)
    return out



# revision 7
# speedup vs baseline: 1.2766x; 1.2766x over previous
"""Routed-LoRA linear layer (moe_routing) on 8 trn2 NeuronCores.

Math (per token t):
  out[t, :] = W @ x[t] + b + 2.0 * sum_n mask[n, t] * (B_n @ (A_n @ x[t]))

Strategy (v2: fp8 DoubleRow main path):
  - Data-parallel over B*T = 65536 tokens: 8192 tokens per core.
  - The main matmul runs in fp8(e4m3) DoubleRow mode (K=256 per
    instruction, 0.5 PE cycles per output row) as a 3-term residual
    compensation at a single product scale of 2^6:
      t1: Q8(x) @ Q8(W*64)            [x_hi  @ W_hi6]
      t2: Q8((x-x_hi)*32) @ Q8(W*2)   [x_lo5 @ W_hi1]
      t3: Q8(x) @ Q8(W*64 - W_hi6)    [x_hi  @ W_lo6]
    All terms land at scale 2^6 in one fp32 PSUM accumulation group, so
    no device-side rescale is needed: the host divides the f32 output by
    64 (exact) and adds the bias in f32. Max-rel error ~9e-3 with dense
    masks (~3e-3 with one-hot), well inside the 2e-2 gate, at 25% less
    PE time than an all-bf16 main matmul would need -- and 4x less than
    the previous bf16 kernel's per-term cost.
  - LoRA: s.T = (A*64 fp8) @ x_hi computed directly in rank-partition
    layout [64, 512] per supertile (4 DoubleRow matmuls, no PE
    transpose), masked on DVE with a host-expanded per-(rank,token)
    bf16 mask (x SCALING), and accumulated into the base matmul's PSUM
    bank as a final bf16 K=64 contraction chunk.
  - Epilogue is a bare PSUM->SBUF bf16 copy (alternating DVE /
    Activation) + DMA; the 1/64 unscale and the bias ride on the host.
  - Preloads are spread across the scalar/gpsimd/vector DMA queues so
    supertile 0 can start as soon as the first (x,W) chunk pair lands;
    supertile 0 runs k-outer across six half-tiles to consume the
    preload stream at its delivery rate.
"""

import numpy as np
import ml_dtypes

import concourse.bass as bass
from concourse import bacc
import concourse.mybir as mybir
import concourse.tile as tile
from concourse.bass_utils import run_bass_kernel_spmd

N_CORES = 8
B, T = 8, 8192
D_IN = 1024
D_OUT = 1024
N_ADAPT, R = 4, 16
NR = N_ADAPT * R  # 64
SCALING = 32.0 / 16.0

TOK = B * T // N_CORES  # 8192 tokens per core
SUP = 512               # tokens per supertile
N_SUP = TOK // SUP      # 16
SUB = 128               # tokens per matmul M-tile
N_SUB = SUP // SUB      # 4
P = 128
KC = D_IN // P          # 8 contraction chunks of 128
NPAIR = KC // 2         # 4 DoubleRow chunk-pairs of 256
NB = D_OUT // 512       # 2 PSUM-bank column halves
S6 = 64.0               # product scale 2^6

F32 = mybir.dt.float32
BF16 = mybir.dt.bfloat16
F8 = mybir.dt.float8e4
NP_BF16 = ml_dtypes.bfloat16
NP_F8 = ml_dtypes.float8_e4m3
DR = mybir.MatmulPerfMode.DoubleRow


def build_bass():
    nc = bacc.Bacc(
        "TRN2", target_bir_lowering=False, debug=False, num_devices=N_CORES
    )

    xhi_d = nc.dram_tensor("xhi", [D_IN, TOK], F8, kind="ExternalInput")
    xlo_d = nc.dram_tensor("xlo", [D_IN, TOK], F8, kind="ExternalInput")
    w6_d = nc.dram_tensor("whi6", [D_IN, D_OUT], F8, kind="ExternalInput")
    w1_d = nc.dram_tensor("whi1", [D_IN, D_OUT], F8, kind="ExternalInput")
    wl_d = nc.dram_tensor("wlo6", [D_IN, D_OUT], F8, kind="ExternalInput")
    a8_d = nc.dram_tensor("a8", [P, KC * NR], F8, kind="ExternalInput")
    bt_d = nc.dram_tensor("btr", [NR, D_OUT], BF16, kind="ExternalInput")
    mj_d = nc.dram_tensor("mj", [NR, TOK], BF16, kind="ExternalInput")
    out_d = nc.dram_tensor("out", [TOK, D_OUT], BF16, kind="ExternalOutput")

    xhi_r = xhi_d.ap().rearrange("(kc p) t -> p kc t", p=P)
    xlo_r = xlo_d.ap().rearrange("(kc p) t -> p kc t", p=P)
    w6_r = w6_d.ap().rearrange("(kc p) n -> p kc n", p=P)
    w1_r = w1_d.ap().rearrange("(kc p) n -> p kc n", p=P)
    wl_r = wl_d.ap().rearrange("(kc p) n -> p kc n", p=P)
    out_r = out_d.ap().rearrange("(s q p) n -> s q p n", q=N_SUB, p=P)

    with tile.TileContext(nc) as tc:
        with (
            tc.tile_pool(name="const", bufs=1) as const,
            tc.tile_pool(name="xhp", bufs=3) as xhp,
            tc.tile_pool(name="xlp", bufs=3) as xlp,
            tc.tile_pool(name="smtp", bufs=2) as smtp,
            tc.tile_pool(name="op", bufs=6) as op,
            tc.tile_pool(name="pso", bufs=6, space="PSUM") as pso,
            tc.tile_pool(name="pst", bufs=2, space="PSUM") as pst,
        ):
            w6_sb = const.tile([P, KC, D_OUT], F8)
            w1_sb = const.tile([P, KC, D_OUT], F8)
            wl_sb = const.tile([P, KC, D_OUT], F8)
            a_sb = const.tile([P, KC, NR], F8)
            bt_sb = const.tile([NR, D_OUT], BF16)
            mj_sb = const.tile([NR, TOK], BF16)

            # scalar queue: A first (sT can start early), then W_hi6 in
            # chunk-pairs (t1 k-outer consumes pairs at delivery rate),
            # then LoRA-B (needed ~7us in), then W_lo6 (t3, ~9us in).
            nc.scalar.dma_start(
                out=a_sb[:],
                in_=a8_d.ap().rearrange("p (kc j) -> p kc j", kc=KC),
            )
            for c in range(NPAIR):
                nc.scalar.dma_start(
                    out=w6_sb[:, 2 * c : 2 * c + 2, :],
                    in_=w6_r[:, 2 * c : 2 * c + 2, :],
                )
            nc.scalar.dma_start(out=bt_sb[:], in_=bt_d.ap())
            nc.scalar.dma_start(out=wl_sb[:], in_=wl_r[:])
            # gpsimd queue: first mask slice (needed ~5us), W_hi1 (t2,
            # ~6us), the rest of the mask (supertile 1+ and 8+)
            nc.gpsimd.dma_start(out=mj_sb[:, :SUP], in_=mj_d.ap()[:, :SUP])
            nc.gpsimd.dma_start(out=w1_sb[:], in_=w1_r[:])
            nc.gpsimd.dma_start(
                out=mj_sb[:, SUP : 8 * SUP], in_=mj_d.ap()[:, SUP : 8 * SUP]
            )
            nc.gpsimd.dma_start(
                out=mj_sb[:, 8 * SUP :], in_=mj_d.ap()[:, 8 * SUP :]
            )

            def mm(ops_t, x_sb, w_sb, c, ts, nsl, start=False, stop=False):
                nc.tensor.matmul(
                    ops_t[:],
                    x_sb[:, 2 * c : 2 * c + 2, ts : ts + SUB],
                    w_sb[:, 2 * c : 2 * c + 2, nsl],
                    start=start,
                    stop=stop,
                    perf_mode=DR,
                )

            for s in range(N_SUP):
                t0 = s * SUP
                xh = xhp.tile([P, KC, SUP], F8, tag="xh")
                xl = xlp.tile([P, KC, SUP], F8, tag="xl")
                if s == 0:
                    for c in range(NPAIR):
                        nc.sync.dma_start(
                            out=xh[:, 2 * c : 2 * c + 2, :],
                            in_=xhi_r[:, 2 * c : 2 * c + 2, t0 : t0 + SUP],
                        )
                else:
                    nc.sync.dma_start(out=xh[:], in_=xhi_r[:, :, t0 : t0 + SUP])
                nc.sync.dma_start(out=xl[:], in_=xlo_r[:, :, t0 : t0 + SUP])

                ps_t = pst.tile([NR, SUP], F32, tag="pst", name=f"pst{s}")
                smt = smtp.tile([NR, SUP], BF16, tag="smt", name=f"smt{s}")

                def sT():
                    # s.T = (A*64).T-stationary @ x_hi: rank-partition
                    # layout directly, no PE transpose needed
                    for c in range(NPAIR):
                        nc.tensor.matmul(
                            ps_t[:],
                            a_sb[:, 2 * c : 2 * c + 2, :],
                            xh[:, 2 * c : 2 * c + 2, :],
                            start=(c == 0),
                            stop=(c == NPAIR - 1),
                            perf_mode=DR,
                        )

                def mask_mul():
                    nc.vector.tensor_mul(
                        smt[:], ps_t[:], mj_sb[:, t0 : t0 + SUP]
                    )

                def lora_store(q, n, ops_t):
                    ts = q * SUB
                    nsl = slice(n * 512, (n + 1) * 512)
                    nc.tensor.matmul(
                        ops_t[:],
                        smt[:, ts : ts + SUB],
                        bt_sb[:, nsl],
                        start=False,
                        stop=True,
                    )
                    # PSUM can't be DMA'd directly; stage through SBUF.
                    # Alternate the copy between DVE and Activation so
                    # neither engine exceeds ~25% busy.
                    o_sb = op.tile([P, 512], BF16, tag="o")
                    if n == 0:
                        nc.vector.tensor_copy(o_sb[:], ops_t[:])
                    else:
                        nc.scalar.activation(
                            o_sb[:], ops_t[:], mybir.ActivationFunctionType.Copy
                        )
                    nc.scalar.dma_start(out=out_r[s, q][:, nsl], in_=o_sb[:])

                if s == 0:
                    # k-outer across six half-tiles: consume each arriving
                    # (x-pair, W-pair) as the preload stream delivers it
                    ph = {}
                    for q in range(3):
                        for n in range(NB):
                            ph[q, n] = pso.tile(
                                [P, 512], F32, tag="ops", name=f"ops0_{q}_{n}"
                            )
                    for c in range(NPAIR):
                        for q in range(3):
                            for n in range(NB):
                                mm(ph[q, n], xh, w6_sb, c, q * SUB,
                                   slice(n * 512, (n + 1) * 512),
                                   start=(c == 0))
                    sT()
                    mask_mul()
                    for c in range(NPAIR):
                        for q in range(3):
                            for n in range(NB):
                                mm(ph[q, n], xl, w1_sb, c, q * SUB,
                                   slice(n * 512, (n + 1) * 512))
                    for c in range(NPAIR):
                        for q in range(3):
                            for n in range(NB):
                                mm(ph[q, n], xh, wl_sb, c, q * SUB,
                                   slice(n * 512, (n + 1) * 512))
                    for q in range(3):
                        for n in range(NB):
                            lora_store(q, n, ph[q, n])
                    q_range = (3,)
                else:
                    sT()
                    mask_mul()
                    q_range = range(N_SUB)

                for q in q_range:
                    ts = q * SUB
                    ops = {}
                    for n in range(NB):
                        ops[n] = pso.tile(
                            [P, 512], F32, tag="ops", name=f"ops{s}_{q}_{n}"
                        )
                        nsl = slice(n * 512, (n + 1) * 512)
                        for c in range(NPAIR):
                            mm(ops[n], xh, w6_sb, c, ts, nsl, start=(c == 0))
                        for c in range(NPAIR):
                            mm(ops[n], xl, w1_sb, c, ts, nsl)
                        for c in range(NPAIR):
                            mm(ops[n], xh, wl_sb, c, ts, nsl)
                    for n in range(NB):
                        lora_store(q, n, ops[n])

    nc.compile()
    return nc


_NC_CACHE = None


def _get_nc():
    global _NC_CACHE
    if _NC_CACHE is None:
        _NC_CACHE = build_bass()
    return _NC_CACHE


def make_in_maps(x, W, b, lora_A, lora_B, masks):
    x = np.ascontiguousarray(x, dtype=np.float32)
    W = np.ascontiguousarray(W, dtype=np.float32)
    lora_A = np.ascontiguousarray(lora_A, dtype=np.float32)
    lora_B = np.ascontiguousarray(lora_B, dtype=np.float32)
    masks = np.ascontiguousarray(masks, dtype=np.float32)

    x_flat = x.reshape(B * T, D_IN)
    A_flat = lora_A.reshape(NR, D_IN)
    B_flat = lora_B.transpose(1, 0, 2).reshape(D_OUT, NR)

    # fp8 residual split of x (shared across cores, then sliced)
    x_hi8 = x_flat.astype(NP_F8)
    x_hi32 = x_hi8.astype(np.float32)
    x_lo8 = ((x_flat - x_hi32) * 32.0).astype(NP_F8)

    Wt = np.ascontiguousarray(W.T)                    # [D_IN, D_OUT]
    w_hi6 = (Wt * S6).astype(NP_F8)
    w_hi1 = (Wt * 2.0).astype(NP_F8)
    w_lo6 = (Wt * S6 - w_hi6.astype(np.float32)).astype(NP_F8)

    a8_full = (A_flat * S6).astype(NP_F8)             # [NR, D_IN]
    a8 = np.ascontiguousarray(
        a8_full.T.reshape(KC, P, NR).transpose(1, 0, 2).reshape(P, KC * NR)
    )
    btr = np.ascontiguousarray(B_flat.T.astype(NP_BF16))  # [NR, D_OUT]

    # per-(rank, token) mask with the LoRA scaling folded in
    m_full = masks[..., 0].reshape(N_ADAPT, B * T) * np.float32(SCALING)
    mj_full = np.repeat(m_full, R, axis=0)            # [NR, B*T]

    in_maps = []
    for c in range(N_CORES):
        sl = slice(c * TOK, (c + 1) * TOK)
        in_maps.append(
            {
                "xhi": np.ascontiguousarray(x_hi8[sl].T),
                "xlo": np.ascontiguousarray(x_lo8[sl].T),
                "whi6": w_hi6,
                "whi1": w_hi1,
                "wlo6": w_lo6,
                "a8": a8,
                "btr": btr,
                "mj": np.ascontiguousarray(mj_full[:, sl].astype(NP_BF16)),
            }
        )
    return in_maps


def kernel(x, W, b, lora_A, lora_B, masks):
    nc = _get_nc()
    in_maps = make_in_maps(x, W, b, lora_A, lora_B, masks)
    res = run_bass_kernel_spmd(nc, in_maps, core_ids=list(range(N_CORES)))
    out = np.concatenate([r["out"] for r in res.results], axis=0)
    out = out.astype(np.float32) * np.float32(1.0 / S6)
    out += np.asarray(b, dtype=np.float32)[None, :]
    return out.reshape(B, T, D_OUT)


# revision 11
# speedup vs baseline: 1.2916x; 1.0117x over previous
"""Routed-LoRA linear layer (moe_routing) on 8 trn2 NeuronCores.

Math (per token t):
  out[t, :] = W @ x[t] + b + 2.0 * sum_n mask[n, t] * (B_n @ (A_n @ x[t]))

Strategy (v2: fp8 DoubleRow main path):
  - Data-parallel over B*T = 65536 tokens: 8192 tokens per core.
  - The main matmul runs in fp8(e4m3) DoubleRow mode (K=256 per
    instruction, 0.5 PE cycles per output row) as a 3-term residual
    compensation at a single product scale of 2^6:
      t1: Q8(x) @ Q8(W*64)            [x_hi  @ W_hi6]
      t2: Q8((x-x_hi)*32) @ Q8(W*2)   [x_lo5 @ W_hi1]
      t3: Q8(x) @ Q8(W*64 - W_hi6)    [x_hi  @ W_lo6]
    All terms land at scale 2^6 in one fp32 PSUM accumulation group, so
    no device-side rescale is needed: the host divides the f32 output by
    64 (exact) and adds the bias in f32. Max-rel error ~9e-3 with dense
    masks (~3e-3 with one-hot), well inside the 2e-2 gate, at 25% less
    PE time than an all-bf16 main matmul would need -- and 4x less than
    the previous bf16 kernel's per-term cost.
  - LoRA: s.T = (A*64 fp8) @ x_hi computed directly in rank-partition
    layout [64, 512] per supertile (4 DoubleRow matmuls, no PE
    transpose), masked on DVE with a host-expanded per-(rank,token)
    bf16 mask (x SCALING), and accumulated into the base matmul's PSUM
    bank as a final bf16 K=64 contraction chunk.
  - Epilogue is a bare PSUM->SBUF bf16 copy (alternating DVE /
    Activation) + DMA; the 1/64 unscale and the bias ride on the host.
  - Preloads are spread across the scalar/gpsimd/vector DMA queues so
    supertile 0 can start as soon as the first (x,W) chunk pair lands;
    supertile 0 runs k-outer across six half-tiles to consume the
    preload stream at its delivery rate.
"""

import numpy as np
import ml_dtypes

import concourse.bass as bass
from concourse import bacc
import concourse.mybir as mybir
import concourse.tile as tile
from concourse.bass_utils import run_bass_kernel_spmd

N_CORES = 8
B, T = 8, 8192
D_IN = 1024
D_OUT = 1024
N_ADAPT, R = 4, 16
NR = N_ADAPT * R  # 64
SCALING = 32.0 / 16.0

TOK = B * T // N_CORES  # 8192 tokens per core
SUP = 512               # tokens per supertile
N_SUP = TOK // SUP      # 16
SUB = 128               # tokens per matmul M-tile
N_SUB = SUP // SUB      # 4
P = 128
KC = D_IN // P          # 8 contraction chunks of 128
NPAIR = KC // 2         # 4 DoubleRow chunk-pairs of 256
NB = D_OUT // 512       # 2 PSUM-bank column halves
S6 = 64.0               # product scale 2^6

F32 = mybir.dt.float32
BF16 = mybir.dt.bfloat16
F8 = mybir.dt.float8e4
NP_BF16 = ml_dtypes.bfloat16
NP_F8 = ml_dtypes.float8_e4m3
DR = mybir.MatmulPerfMode.DoubleRow


def build_bass():
    nc = bacc.Bacc(
        "TRN2", target_bir_lowering=False, debug=False, num_devices=N_CORES
    )

    xhi_d = nc.dram_tensor("xhi", [D_IN, TOK], F8, kind="ExternalInput")
    xlo_d = nc.dram_tensor("xlo", [D_IN, TOK], F8, kind="ExternalInput")
    w6_d = nc.dram_tensor("whi6", [D_IN, D_OUT], F8, kind="ExternalInput")
    w1_d = nc.dram_tensor("whi1", [D_IN, D_OUT], F8, kind="ExternalInput")
    wl_d = nc.dram_tensor("wlo6", [D_IN, D_OUT], F8, kind="ExternalInput")
    a8_d = nc.dram_tensor("a8", [P, KC * NR], F8, kind="ExternalInput")
    bt_d = nc.dram_tensor("btr", [NR, D_OUT], BF16, kind="ExternalInput")
    mj_d = nc.dram_tensor("mj", [NR, TOK], BF16, kind="ExternalInput")
    out_d = nc.dram_tensor("out", [TOK, D_OUT], BF16, kind="ExternalOutput")

    xhi_r = xhi_d.ap().rearrange("(kc p) t -> p kc t", p=P)
    xlo_r = xlo_d.ap().rearrange("(kc p) t -> p kc t", p=P)
    w6_r = w6_d.ap().rearrange("(kc p) n -> p kc n", p=P)
    w1_r = w1_d.ap().rearrange("(kc p) n -> p kc n", p=P)
    wl_r = wl_d.ap().rearrange("(kc p) n -> p kc n", p=P)
    out_r = out_d.ap().rearrange("(s q p) n -> s q p n", q=N_SUB, p=P)

    with tile.TileContext(nc) as tc:
        with (
            tc.tile_pool(name="const", bufs=1) as const,
            tc.tile_pool(name="xhp", bufs=3) as xhp,
            tc.tile_pool(name="xlp", bufs=3) as xlp,
            tc.tile_pool(name="smtp", bufs=2) as smtp,
            tc.tile_pool(name="op", bufs=6) as op,
            tc.tile_pool(name="pso", bufs=6, space="PSUM") as pso,
            tc.tile_pool(name="pst", bufs=2, space="PSUM") as pst,
        ):
            w6_sb = const.tile([P, KC, D_OUT], F8)
            w1_sb = const.tile([P, KC, D_OUT], F8)
            wl_sb = const.tile([P, KC, D_OUT], F8)
            a_sb = const.tile([P, KC, NR], F8)
            bt_sb = const.tile([NR, D_OUT], BF16)
            mj_sb = const.tile([NR, TOK], BF16)
            warm_sb = const.tile([P, 272], F8)

            # PE p-state warmup: the tensor engine ramps 0.65 -> 1.2 ->
            # 2.4 GHz over ~3us of continuous work, and the first real
            # matmul can't start until the first DMAs land (~3.8us).
            # Burn the ramp on zero matmuls so real work runs full-speed.
            nc.vector.memset(warm_sb[:], 0.0)
            warm_ps = pso.tile([P, 512], F32, tag="ops", name="warm")
            for i in range(16):
                nc.tensor.matmul(
                    warm_ps[:16, :256],
                    warm_sb[:, 0:16],
                    warm_sb[:, 16:272],
                    start=True,
                    stop=True,
                )

            # Each DMA carries ~2.3us of serialized queue overhead, so
            # preloads are few and big, spread across all three queues.
            # scalar: W_hi6 in two halves (t1 wave A can start on the
            # first), then A (sT, ~10us in), then LoRA-B (~14us).
            nc.scalar.dma_start(out=w6_sb[:, 0:4, :], in_=w6_r[:, 0:4, :])
            nc.scalar.dma_start(out=w6_sb[:, 4:8, :], in_=w6_r[:, 4:8, :])
            nc.scalar.dma_start(
                out=a_sb[:],
                in_=a8_d.ap().rearrange("p (kc j) -> p kc j", kc=KC),
            )
            nc.scalar.dma_start(out=bt_sb[:], in_=bt_d.ap())
            # gpsimd (SWDGE): W_hi1 (t2, ~7us), W_lo6 (t3, ~11us), mask
            nc.gpsimd.dma_start(out=w1_sb[:], in_=w1_r[:])
            nc.gpsimd.dma_start(out=wl_sb[:], in_=wl_r[:])
            nc.gpsimd.dma_start(out=mj_sb[:, :SUP], in_=mj_d.ap()[:, :SUP])
            nc.gpsimd.dma_start(
                out=mj_sb[:, SUP : 8 * SUP], in_=mj_d.ap()[:, SUP : 8 * SUP]
            )
            nc.gpsimd.dma_start(
                out=mj_sb[:, 8 * SUP :], in_=mj_d.ap()[:, 8 * SUP :]
            )

            def mm(ops_t, x_sb, w_sb, c, ts, nsl, start=False, stop=False):
                nc.tensor.matmul(
                    ops_t[:],
                    x_sb[:, 2 * c : 2 * c + 2, ts : ts + SUB],
                    w_sb[:, 2 * c : 2 * c + 2, nsl],
                    start=start,
                    stop=stop,
                    perf_mode=DR,
                )

            for s in range(N_SUP):
                t0 = s * SUP
                xh = xhp.tile([P, KC, SUP], F8, tag="xh")
                xl = xlp.tile([P, KC, SUP], F8, tag="xl")
                if s == 0:
                    nc.sync.dma_start(
                        out=xh[:, 0:4, :], in_=xhi_r[:, 0:4, t0 : t0 + SUP]
                    )
                    nc.sync.dma_start(
                        out=xh[:, 4:8, :], in_=xhi_r[:, 4:8, t0 : t0 + SUP]
                    )
                else:
                    nc.sync.dma_start(out=xh[:], in_=xhi_r[:, :, t0 : t0 + SUP])
                nc.sync.dma_start(out=xl[:], in_=xlo_r[:, :, t0 : t0 + SUP])

                ps_t = pst.tile([NR, SUP], F32, tag="pst", name=f"pst{s}")
                smt = smtp.tile([NR, SUP], BF16, tag="smt", name=f"smt{s}")

                def sT():
                    # s.T = (A*64).T-stationary @ x_hi: rank-partition
                    # layout directly, no PE transpose needed
                    for c in range(NPAIR):
                        nc.tensor.matmul(
                            ps_t[:],
                            a_sb[:, 2 * c : 2 * c + 2, :],
                            xh[:, 2 * c : 2 * c + 2, :],
                            start=(c == 0),
                            stop=(c == NPAIR - 1),
                            perf_mode=DR,
                        )

                def mask_mul():
                    nc.vector.tensor_mul(
                        smt[:], ps_t[:], mj_sb[:, t0 : t0 + SUP]
                    )

                def lora_store(q, n, ops_t):
                    ts = q * SUB
                    nsl = slice(n * 512, (n + 1) * 512)
                    nc.tensor.matmul(
                        ops_t[:],
                        smt[:, ts : ts + SUB],
                        bt_sb[:, nsl],
                        start=False,
                        stop=True,
                    )
                    # PSUM can't be DMA'd directly; stage through SBUF.
                    # Alternate the copy between DVE and Activation so
                    # neither engine exceeds ~25% busy.
                    o_sb = op.tile([P, 512], BF16, tag="o")
                    if n == 0:
                        nc.vector.tensor_copy(o_sb[:], ops_t[:])
                    else:
                        nc.scalar.activation(
                            o_sb[:], ops_t[:], mybir.ActivationFunctionType.Copy
                        )
                    nc.scalar.dma_start(out=out_r[s, q][:, nsl], in_=o_sb[:])

                if s == 0:
                    # k-outer across six half-tiles in term-waves, each
                    # wave gated on one big preload DMA: t1 wave A (W_hi6
                    # first half), t1 wave B (second half), t2 (W_hi1),
                    # sT (A), t3 (W_lo6), so PE never waits long.
                    ph = {}
                    for q in range(3):
                        for n in range(NB):
                            ph[q, n] = pso.tile(
                                [P, 512], F32, tag="ops", name=f"ops0_{q}_{n}"
                            )
                    for cs in ((0, 1), (2, 3)):
                        for c in cs:
                            for q in range(3):
                                for n in range(NB):
                                    mm(ph[q, n], xh, w6_sb, c, q * SUB,
                                       slice(n * 512, (n + 1) * 512),
                                       start=(c == 0))
                    for c in range(NPAIR):
                        for q in range(3):
                            for n in range(NB):
                                mm(ph[q, n], xl, w1_sb, c, q * SUB,
                                   slice(n * 512, (n + 1) * 512))
                    sT()
                    mask_mul()
                    for c in range(NPAIR):
                        for q in range(3):
                            for n in range(NB):
                                mm(ph[q, n], xh, wl_sb, c, q * SUB,
                                   slice(n * 512, (n + 1) * 512))
                    for q in range(3):
                        for n in range(NB):
                            lora_store(q, n, ph[q, n])
                    q_range = (3,)
                else:
                    sT()
                    mask_mul()
                    q_range = range(N_SUB)

                for q in q_range:
                    ts = q * SUB
                    last = s == N_SUP - 1 and q == N_SUB - 1

                    def mains(n):
                        t = pso.tile(
                            [P, 512], F32, tag="ops", name=f"ops{s}_{q}_{n}"
                        )
                        nsl = slice(n * 512, (n + 1) * 512)
                        for c in range(NPAIR):
                            mm(t, xh, w6_sb, c, ts, nsl, start=(c == 0))
                        for c in range(NPAIR):
                            mm(t, xl, w1_sb, c, ts, nsl)
                        for c in range(NPAIR):
                            mm(t, xh, wl_sb, c, ts, nsl)
                        return t

                    if not last:
                        ops = {n: mains(n) for n in range(NB)}
                        for n in range(NB):
                            lora_store(q, n, ops[n])
                    else:
                        # Final tile: drain half 1 while half 0's mains
                        # run, then split half 0's copy across DVE +
                        # Activation and its store across both queues to
                        # shorten the end-of-kernel tail.
                        ops1 = mains(1)
                        nsl1 = slice(512, 1024)
                        nc.tensor.matmul(
                            ops1[:], smt[:, ts : ts + SUB], bt_sb[:, nsl1],
                            start=False, stop=True,
                        )
                        o1 = op.tile([P, 512], BF16, tag="o")
                        nc.vector.tensor_copy(o1[:], ops1[:])
                        nc.scalar.dma_start(out=out_r[s, q][:, nsl1], in_=o1[:])
                        ops0 = mains(0)
                        nc.tensor.matmul(
                            ops0[:], smt[:, ts : ts + SUB], bt_sb[:, 0:512],
                            start=False, stop=True,
                        )
                        o0 = op.tile([P, 512], BF16, tag="o")
                        nc.vector.tensor_copy(o0[:, 0:256], ops0[:, 0:256])
                        nc.scalar.activation(
                            o0[:, 256:512], ops0[:, 256:512],
                            mybir.ActivationFunctionType.Copy,
                        )
                        nc.scalar.dma_start(
                            out=out_r[s, q][:, 0:256], in_=o0[:, 0:256]
                        )
                        nc.sync.dma_start(
                            out=out_r[s, q][:, 256:512], in_=o0[:, 256:512]
                        )

    nc.compile()
    return nc


_NC_CACHE = None


def _get_nc():
    global _NC_CACHE
    if _NC_CACHE is None:
        _NC_CACHE = build_bass()
    return _NC_CACHE


def make_in_maps(x, W, b, lora_A, lora_B, masks):
    x = np.ascontiguousarray(x, dtype=np.float32)
    W = np.ascontiguousarray(W, dtype=np.float32)
    lora_A = np.ascontiguousarray(lora_A, dtype=np.float32)
    lora_B = np.ascontiguousarray(lora_B, dtype=np.float32)
    masks = np.ascontiguousarray(masks, dtype=np.float32)

    x_flat = x.reshape(B * T, D_IN)
    A_flat = lora_A.reshape(NR, D_IN)
    B_flat = lora_B.transpose(1, 0, 2).reshape(D_OUT, NR)

    # fp8 residual split of x (shared across cores, then sliced)
    x_hi8 = x_flat.astype(NP_F8)
    x_hi32 = x_hi8.astype(np.float32)
    x_lo8 = ((x_flat - x_hi32) * 32.0).astype(NP_F8)

    Wt = np.ascontiguousarray(W.T)                    # [D_IN, D_OUT]
    w_hi6 = (Wt * S6).astype(NP_F8)
    w_hi1 = (Wt * 2.0).astype(NP_F8)
    w_lo6 = (Wt * S6 - w_hi6.astype(np.float32)).astype(NP_F8)

    a8_full = (A_flat * S6).astype(NP_F8)             # [NR, D_IN]
    a8 = np.ascontiguousarray(
        a8_full.T.reshape(KC, P, NR).transpose(1, 0, 2).reshape(P, KC * NR)
    )
    btr = np.ascontiguousarray(B_flat.T.astype(NP_BF16))  # [NR, D_OUT]

    # per-(rank, token) mask with the LoRA scaling folded in
    m_full = masks[..., 0].reshape(N_ADAPT, B * T) * np.float32(SCALING)
    mj_full = np.repeat(m_full, R, axis=0)            # [NR, B*T]

    in_maps = []
    for c in range(N_CORES):
        sl = slice(c * TOK, (c + 1) * TOK)
        in_maps.append(
            {
                "xhi": np.ascontiguousarray(x_hi8[sl].T),
                "xlo": np.ascontiguousarray(x_lo8[sl].T),
                "whi6": w_hi6,
                "whi1": w_hi1,
                "wlo6": w_lo6,
                "a8": a8,
                "btr": btr,
                "mj": np.ascontiguousarray(mj_full[:, sl].astype(NP_BF16)),
            }
        )
    return in_maps


def kernel(x, W, b, lora_A, lora_B, masks):
    nc = _get_nc()
    in_maps = make_in_maps(x, W, b, lora_A, lora_B, masks)
    res = run_bass_kernel_spmd(nc, in_maps, core_ids=list(range(N_CORES)))
    out = np.concatenate([r["out"] for r in res.results], axis=0)
    out = out.astype(np.float32) * np.float32(1.0 / S6)
    out += np.asarray(b, dtype=np.float32)[None, :]
    return out.reshape(B, T, D_OUT)
